# revision 1
# baseline (speedup 1.0000x reference)
"""Bass/Trainium2 kernel for a 2-layer multi-head GAT (DocRE model).

Contract: kernel(**inputs) takes the FULL unsharded inputs as numpy arrays
and returns the FULL [512, 768] float32 output. Internally the 512 nodes are
row-sharded across 8 NeuronCores; per-head weights are replicated; the small
x1 / h1 activations are AllGathered on-device between the two layers.

Key layout decisions:
- e is pre-transposed on the host to [i, k, j] and cast to bf16 so the big
  edge-score contraction streams at full HBM bandwidth with features on the
  partition axis (no on-device transposes of e).
- Both layers' edge scores come from ONE pass over e using the host-folded
  [768, 24] matrix V = [We0(W0 a0_3) | We1(W1 a1_3)].
- Scores for 4 rows are packed into one [128, 512] tile via PE column tiling
  (4 groups of 24 partition rows); softmax runs without max-subtraction.
- Layer-0 additive score terms (s_src + s_dst + adj mask) depend only on
  inputs -> precomputed on host. Layer-1's are built on device from x1.
- h1 is column-sharded (1152 of 9216 cols per core), AllGathered in bf16.
"""

import sys

sys.path.insert(0, "/opt/trn_rl_repo")

import numpy as np
import ml_dtypes

from concourse import bass, bacc, mybir, tile
from concourse.bass_utils import run_bass_kernel_spmd

BF16 = ml_dtypes.bfloat16

N = 512          # nodes
D = 768          # hidden
H = 12           # heads
F0 = 64          # layer-0 per-head dim
NCORES = 8
NPC = N // NCORES          # 64 local rows per core
NBLK = NPC // 4            # 16 blocks of 4 rows
ALPHA = 0.2
KT = D // 128              # 6 contraction tiles
W1COLS = H * D // NCORES   # 1152 h1 columns per core

F32 = mybir.dt.float32
BF = mybir.dt.bfloat16
ADD = mybir.AluOpType.add
MULT = mybir.AluOpType.mult
AF = mybir.ActivationFunctionType

_COMPILED = None
DEBUG = False
SIM_SAFE = False  # replace Prelu (not in interp) with Relu for cost-model sims
_LAST_RESULTS = None


def _build_nc():
    nc = bacc.Bacc("TRN2", target_bir_lowering=False, num_devices=NCORES)
    lrelu_fn = AF.Relu if SIM_SAFE else AF.Prelu
    dbg = {}
    if DEBUG:
        dbg["sc0"] = nc.dram_tensor("dbg_sc0", [128, N], F32, kind="ExternalOutput")
        dbg["att0"] = nc.dram_tensor("dbg_att0", [128, N], F32, kind="ExternalOutput")
        dbg["x1"] = nc.dram_tensor("dbg_x1", [NPC, D], F32, kind="ExternalOutput")
        dbg["s1"] = nc.dram_tensor("dbg_s1", [24, N], F32, kind="ExternalOutput")
        dbg["s1loc"] = nc.dram_tensor("dbg_s1loc", [24, NPC], F32, kind="ExternalOutput")
        dbg["sc1"] = nc.dram_tensor("dbg_sc1", [128, N], F32, kind="ExternalOutput")
        dbg["h1"] = nc.dram_tensor("dbg_h1", [128, W1COLS], F32, kind="ExternalOutput")

    eT_d = nc.dram_tensor("eT", [NPC, D, N], BF, kind="ExternalInput")
    xT_d = nc.dram_tensor("xT", [128, KT * N], BF, kind="ExternalInput")
    w0r_d = nc.dram_tensor("w0r", [128, KT * D], BF, kind="ExternalInput")
    w1s_d = nc.dram_tensor("w1s", [128, KT * W1COLS], BF, kind="ExternalInput")
    v_d = nc.dram_tensor("vw", [128, KT * 32], BF, kind="ExternalInput")
    u1_d = nc.dram_tensor("u1", [128, KT * 24], BF, kind="ExternalInput")
    ha0_d = nc.dram_tensor("ha0", [NBLK, 128, N], BF, kind="ExternalInput")
    mk1_d = nc.dram_tensor("mk1", [NBLK, 128, N], BF, kind="ExternalInput")
    ident_d = nc.dram_tensor("ident", [128, 128], BF, kind="ExternalInput")

    out_d = nc.dram_tensor("out", [NPC, D], F32, kind="ExternalOutput")

    agx_in = nc.dram_tensor("agx_in", [NPC, D], BF)
    agx_out = nc.dram_tensor("agx_out", [N, D], BF, addr_space="Shared")
    agh_in = nc.dram_tensor("agh_in", [N, W1COLS], BF)
    agh_out = nc.dram_tensor("agh_out", [N * NCORES, W1COLS], BF, addr_space="Shared")

    with tile.TileContext(nc) as tc:
        with (
            tc.tile_pool(name="const", bufs=1) as constp,
            tc.tile_pool(name="pers", bufs=1) as pers,
            tc.tile_pool(name="hapool", bufs=3) as hapool,
        ):
            ident = constp.tile([128, 128], BF, tag="ident")
            nc.sync.dma_start(out=ident[:, :], in_=ident_d[:, :])
            w1s = constp.tile([128, KT * W1COLS], BF, tag="w1s")
            nc.sync.dma_start(out=w1s[:, :], in_=w1s_d[:, :])
            u1 = constp.tile([128, KT * 24], BF, tag="u1")
            nc.sync.dma_start(out=u1[:, :], in_=u1_d[:, :])

            se1st = [pers.tile([128, N], F32, tag=f"se1_{b}", name=f"se1_{b}") for b in range(NBLK)]
            x1T = pers.tile([128, KT * N], BF, tag="x1T")
            s1loc = pers.tile([24, NPC], F32, tag="s1loc")
            dsta1 = pers.tile([128, N], F32, tag="dsta1")
            src1c = pers.tile([128, NBLK], F32, tag="src1c")

            # =================== phase A+B+C: layer 0 ===================
            with (
                tc.tile_pool(name="l0pers", bufs=1) as l0p,
                tc.tile_pool(name="l0const", bufs=1) as l0c,
                tc.tile_pool(name="epool", bufs=2) as epool,
                tc.tile_pool(name="l0work", bufs=2) as work,
            ):
                xT = l0c.tile([128, KT * N], BF, tag="xT")
                nc.sync.dma_start(out=xT[:, :], in_=xT_d[:, :])
                w0r = l0c.tile([128, KT * D], BF, tag="w0r")
                nc.sync.dma_start(out=w0r[:, :], in_=w0r_d[:, :])
                vw = l0c.tile([128, KT * 32], BF, tag="vw")
                nc.sync.dma_start(out=vw[:, :], in_=v_d[:, :])

                # ---- h0 = x @ W0r -> [4][128 nodes, 768] bf16 ----
                h0 = [l0p.tile([128, D], BF, tag=f"h0_{m}", name=f"h0_{m}") for m in range(4)]
                with tc.tile_pool(name="psh0", bufs=2, space="PSUM") as psh0:
                    for m in range(4):
                        pa = psh0.tile([128, 512], F32, tag="ph0a")
                        pb = psh0.tile([128, 256], F32, tag="ph0b")
                        for k in range(KT):
                            lhs = xT[:, k * N + 128 * m : k * N + 128 * (m + 1)]
                            nc.tensor.matmul(
                                pa[:, :], lhs, w0r[:, k * D : k * D + 512],
                                start=(k == 0), stop=(k == KT - 1),
                            )
                            nc.tensor.matmul(
                                pb[:, :], lhs, w0r[:, k * D + 512 : (k + 1) * D],
                                start=(k == 0), stop=(k == KT - 1),
                            )
                        nc.vector.tensor_copy(out=h0[m][:, 0:512], in_=pa[:, :])
                        nc.vector.tensor_copy(out=h0[m][:, 512:768], in_=pb[:, :])

                # ---- e-pass: scores + softmax + att0^T, 16 blocks ----
                at0T = [
                    l0p.tile([128, NBLK * 128], BF, tag=f"at0T_{q}", name=f"at0T_{q}") for q in range(4)
                ]
                with (
                    tc.tile_pool(name="psv", bufs=2, space="PSUM") as psvp,
                    tc.tile_pool(name="pst", bufs=2, space="PSUM") as pstp,
                ):
                    for b in range(NBLK):
                        et = epool.tile([128, KT * 4 * N], BF, tag="etile")
                        nc.sync.dma_start(
                            out=et[:, :].rearrange(
                                "p (c kb j) -> p c kb j", kb=KT, c=4
                            ),
                            in_=eT_d[4 * b : 4 * b + 4].rearrange(
                                "c (kb p) j -> p c kb j", p=128
                            ),
                        )
                        ha = hapool.tile([128, N], BF, tag="ha0")
                        nc.scalar.dma_start(out=ha[:, :], in_=ha0_d[b])

                        psv = psvp.tile([128, N], F32, tag="psv")
                        for cc in range(4):
                            for k in range(KT):
                                nc.tensor.matmul(
                                    psv[32 * cc : 32 * cc + 32, :],
                                    vw[:, 32 * k : 32 * (k + 1)],
                                    et[:, (cc * KT + k) * N : (cc * KT + k + 1) * N],
                                    start=(k == 0), stop=(k == KT - 1),
                                    tile_position=(0, 32 * cc),
                                )
                        nc.scalar.copy(out=se1st[b][:, :], in_=psv[:, :])
                        sc0 = work.tile([128, N], F32, tag="sc0")
                        nc.vector.tensor_tensor(
                            out=sc0[:, :], in0=psv[:, :], in1=ha[:, :], op=ADD
                        )
                        lr0 = work.tile([128, N], F32, tag="lr0")
                        nc.scalar.activation(
                            lr0[:, :], sc0[:, :], lrelu_fn, alpha=ALPHA
                        )
                        ex0 = work.tile([128, N], F32, tag="ex0")
                        z0 = work.tile([128, 1], F32, tag="z0")
                        nc.scalar.activation(
                            ex0[:, :], lr0[:, :], AF.Exp, accum_out=z0[:, :]
                        )
                        rz0 = work.tile([128, 1], F32, tag="rz0")
                        nc.vector.reciprocal(rz0[:, :], z0[:, :])
                        if DEBUG and b == 0:
                            nc.sync.dma_start(out=dbg["sc0"][:, :], in_=sc0[:, :])
                        at0 = work.tile([128, N], BF, tag="at0")
                        nc.vector.tensor_scalar(
                            out=at0[:, :], in0=ex0[:, :], scalar1=rz0[:, :],
                            scalar2=None, op0=MULT,
                        )
                        if DEBUG and b == 0:
                            at0f = work.tile([128, N], F32, tag="at0f", bufs=1)
                            nc.vector.tensor_copy(out=at0f[:, :], in_=at0[:, :])
                            nc.sync.dma_start(out=dbg["att0"][:, :], in_=at0f[:, :])
                        for q in range(4):
                            pt = pstp.tile([128, 128], BF, tag="ptr")
                            nc.tensor.transpose(
                                pt[:, :], at0[:, 128 * q : 128 * (q + 1)], ident[:, :]
                            )
                            nc.vector.tensor_copy(
                                out=at0T[q][:, 128 * b : 128 * (b + 1)], in_=pt[:, :]
                            )

                # ---- x1 = elu(concat_h att0 @ h0) ----
                x1bf = work.tile([64, D], BF, tag="x1bf", bufs=1)
                with tc.tile_pool(name="psx1", bufs=1, space="PSUM") as psx1:
                    px1a = psx1.tile([64, 512], F32, tag="px1a")
                    px1b = psx1.tile([64, 256], F32, tag="px1b")
                    for h in range(H):
                        dst = (
                            px1a[:, 64 * h : 64 * (h + 1)]
                            if h < 8
                            else px1b[:, 64 * (h - 8) : 64 * (h - 7)]
                        )
                        for q in range(4):
                            lhs = at0T[q][:, :].rearrange(
                                "p (b c r) -> p b c r", b=NBLK, c=4
                            )[:, :, :, h : h + 1]
                            nc.tensor.matmul(
                                dst, lhs, h0[q][:, 64 * h : 64 * (h + 1)],
                                start=(q == 0), stop=(q == 3),
                            )
                    x1p = work.tile([64, D], F32, tag="x1p", bufs=1)
                    nc.vector.tensor_copy(out=x1p[:, 0:512], in_=px1a[:, :])
                    nc.vector.tensor_copy(out=x1p[:, 512:768], in_=px1b[:, :])
                tmin = work.tile([64, D], F32, tag="tmin", bufs=1)
                nc.vector.tensor_scalar(
                    out=tmin[:, :], in0=x1p[:, :], scalar1=0.0, scalar2=None,
                    op0=mybir.AluOpType.min,
                )
                texp = work.tile([64, D], F32, tag="texp", bufs=1)
                nc.scalar.activation(texp[:, :], tmin[:, :], AF.Exp)
                tmax = work.tile([64, D], F32, tag="tmax", bufs=1)
                nc.vector.tensor_scalar(
                    out=tmax[:, :], in0=x1p[:, :], scalar1=0.0, scalar2=None,
                    op0=mybir.AluOpType.max,
                )
                nc.vector.scalar_tensor_tensor(
                    out=x1bf[:, :], in0=texp[:, :], scalar=-1.0, in1=tmax[:, :],
                    op0=ADD, op1=ADD,
                )

                if DEBUG:
                    x1f32 = work.tile([64, D], F32, tag="x1f32", bufs=1)
                    nc.vector.scalar_tensor_tensor(
                        out=x1f32[:, :], in0=texp[:, :], scalar=-1.0, in1=tmax[:, :],
                        op0=ADD, op1=ADD,
                    )
                    nc.sync.dma_start(out=dbg["x1"][:, :], in_=x1f32[:, :])

                # local x1^T for s_src1 (per-core rows, same program on all cores)
                x1locT = l0p.tile([128, KT * NPC], BF, tag="x1locT")
                with tc.tile_pool(name="pslt", bufs=2, space="PSUM") as pslt:
                    for k6 in range(KT):
                        pt = pslt.tile([128, 64], BF, tag="plt")
                        nc.tensor.transpose(
                            pt[:, 0:64],
                            x1bf[:, 128 * k6 : 128 * (k6 + 1)],
                            ident[0:64, 0:64],
                        )
                        nc.vector.tensor_copy(
                            out=x1locT[:, NPC * k6 : NPC * (k6 + 1)], in_=pt[:, 0:64]
                        )
                    psl = pslt.tile([24, NPC], F32, tag="psl")
                    for k in range(KT):
                        nc.tensor.matmul(
                            psl[:, :], u1[:, 24 * k : 24 * (k + 1)],
                            x1locT[:, NPC * k : NPC * (k + 1)],
                            start=(k == 0), stop=(k == KT - 1),
                        )
                    nc.vector.tensor_copy(out=s1loc[:, :], in_=psl[:, :])
                    if DEBUG:
                        nc.sync.dma_start(out=dbg["s1loc"][:, :], in_=s1loc[:, :])

                # ---- AllGather x1 ----
                nc.scalar.dma_start(out=agx_in[:, :], in_=x1bf[:, :])
                nc.gpsimd.collective_compute(
                    "AllGather", mybir.AluOpType.bypass,
                    replica_groups=[list(range(NCORES))],
                    ins=[agx_in.ap().opt()], outs=[agx_out.ap().opt()],
                )
                with tc.tile_pool(name="x1fp", bufs=1) as x1fp:
                    x1f = [x1fp.tile([128, D], BF, tag=f"x1f_{m}", name=f"x1f_{m}") for m in range(4)]
                    for m in range(4):
                        nc.sync.dma_start(
                            out=x1f[m][:, :], in_=agx_out[128 * m : 128 * (m + 1), :]
                        )
                    with tc.tile_pool(name="psxt", bufs=2, space="PSUM") as psxt:
                        for m in range(4):
                            for k6 in range(KT):
                                pt = psxt.tile([128, 128], BF, tag="pxt")
                                nc.tensor.transpose(
                                    pt[:, :],
                                    x1f[m][:, 128 * k6 : 128 * (k6 + 1)],
                                    ident[:, :],
                                )
                                nc.vector.tensor_copy(
                                    out=x1T[
                                        :, N * k6 + 128 * m : N * k6 + 128 * (m + 1)
                                    ],
                                    in_=pt[:, :],
                                )

                # ---- s_dst1 for all nodes ----
                with tc.tile_pool(name="pss1", bufs=1, space="PSUM") as pss1:
                    ps1 = pss1.tile([24, N], F32, tag="ps1")
                    for k in range(KT):
                        nc.tensor.matmul(
                            ps1[:, :], u1[:, 24 * k : 24 * (k + 1)],
                            x1T[:, N * k : N * (k + 1)],
                            start=(k == 0), stop=(k == KT - 1),
                        )
                    s1 = work.tile([24, N], F32, tag="s1", bufs=1)
                    nc.vector.tensor_copy(out=s1[:, :], in_=ps1[:, :])
                    if DEBUG:
                        nc.sync.dma_start(out=dbg["s1"][:, :], in_=s1[:, :])
                    nc.vector.memset(dsta1[:, :], 0.0)
                    nc.vector.memset(src1c[:, :], 0.0)
                    for cc in range(4):
                        nc.sync.dma_start(
                            out=dsta1[32 * cc + 12 : 32 * cc + 24, :],
                            in_=s1[12:24, :],
                        )
                        nc.sync.dma_start(
                            out=src1c[32 * cc + 12 : 32 * cc + 24, :],
                            in_=s1loc[0:12, :].rearrange(
                                "h (b c) -> h b c", c=4
                            )[:, :, cc : cc + 1],
                        )

                # ---- h1 column slice = x1 @ W1r[:, my cols] ----
                widths = [(0, 512), (512, 1024), (1024, 1152)]
                with tc.tile_pool(name="psh1", bufs=2, space="PSUM") as psh1:
                    for m in range(4):
                        ph1 = [
                            psh1.tile([128, 512], F32, tag="ph1a", name="ph1a"),
                            psh1.tile([128, 512], F32, tag="ph1b", name="ph1b"),
                            psh1.tile([128, 128], F32, tag="ph1c", name="ph1c"),
                        ]
                        for k in range(KT):
                            lhs = x1T[:, N * k + 128 * m : N * k + 128 * (m + 1)]
                            for t, (c0, c1) in enumerate(widths):
                                nc.tensor.matmul(
                                    ph1[t][:, 0 : c1 - c0], lhs,
                                    w1s[:, W1COLS * k + c0 : W1COLS * k + c1],
                                    start=(k == 0), stop=(k == KT - 1),
                                )
                        h1m = work.tile([128, W1COLS], BF, tag="h1m")
                        for t, (c0, c1) in enumerate(widths):
                            nc.vector.tensor_copy(
                                out=h1m[:, c0:c1], in_=ph1[t][:, 0 : c1 - c0]
                            )
                        nc.scalar.dma_start(
                            out=agh_in[128 * m : 128 * (m + 1), :], in_=h1m[:, :]
                        )
                        if DEBUG and m == 0:
                            h1dbg = work.tile([128, W1COLS], F32, tag="h1dbg", bufs=1)
                            nc.vector.tensor_copy(out=h1dbg[:, :], in_=h1m[:, :])
                            nc.sync.dma_start(out=dbg["h1"][:, :], in_=h1dbg[:, :])

            # =================== phase D+E: layer 1 ===================
            nc.gpsimd.collective_compute(
                "AllGather", mybir.AluOpType.bypass,
                replica_groups=[list(range(NCORES))],
                ins=[agh_in.ap().opt()], outs=[agh_out.ap().opt()],
            )
            with (
                tc.tile_pool(name="h1fpool", bufs=1) as h1fpool,
                tc.tile_pool(name="l1pers", bufs=1) as l1p,
                tc.tile_pool(name="l1work", bufs=2) as work,
            ):
                h1f = [h1fpool.tile([128, H * D], BF, tag=f"h1f_{q}", name=f"h1f_{q}") for q in range(4)]
                for q in range(4):
                    nc.sync.dma_start(
                        out=h1f[q][:, :].rearrange(
                            "p (s f) -> p s f", s=NCORES
                        ),
                        in_=agh_out[:, :].rearrange(
                            "(s qq p) f -> qq p s f", qq=4, p=128
                        )[q],
                    )

                at1T = [
                    l1p.tile([128, NBLK * 128], BF, tag=f"at1T_{q}", name=f"at1T_{q}") for q in range(4)
                ]
                with tc.tile_pool(name="pst1", bufs=2, space="PSUM") as pstp:
                    for b in range(NBLK):
                        mk = hapool.tile([128, N], BF, tag="mk1")
                        nc.scalar.dma_start(out=mk[:, :], in_=mk1_d[b])
                        t1 = work.tile([128, N], F32, tag="t1")
                        nc.vector.scalar_tensor_tensor(
                            out=t1[:, :], in0=se1st[b][:, :],
                            scalar=src1c[:, b : b + 1], in1=dsta1[:, :],
                            op0=ADD, op1=ADD,
                        )
                        sc1 = work.tile([128, N], F32, tag="sc1")
                        nc.vector.tensor_tensor(
                            out=sc1[:, :], in0=t1[:, :], in1=mk[:, :], op=ADD
                        )
                        if DEBUG and b == 0:
                            nc.sync.dma_start(out=dbg["sc1"][:, :], in_=sc1[:, :])
                        lr1 = work.tile([128, N], F32, tag="lr1")
                        nc.scalar.activation(
                            lr1[:, :], sc1[:, :], lrelu_fn, alpha=ALPHA
                        )
                        ex1 = work.tile([128, N], F32, tag="ex1")
                        z1 = work.tile([128, 1], F32, tag="z1")
                        nc.scalar.activation(
                            ex1[:, :], lr1[:, :], AF.Exp, accum_out=z1[:, :]
                        )
                        rz1 = work.tile([128, 1], F32, tag="rz1")
                        nc.vector.reciprocal(rz1[:, :], z1[:, :])
                        at1 = work.tile([128, N], BF, tag="at1")
                        nc.vector.tensor_scalar(
                            out=at1[:, :], in0=ex1[:, :], scalar1=rz1[:, :],
                            scalar2=None, op0=MULT,
                        )
                        for q in range(4):
                            pt = pstp.tile([128, 128], BF, tag="ptr1")
                            nc.tensor.transpose(
                                pt[:, :], at1[:, 128 * q : 128 * (q + 1)], ident[:, :]
                            )
                            nc.vector.tensor_copy(
                                out=at1T[q][:, 128 * b : 128 * (b + 1)], in_=pt[:, :]
                            )

                # ---- output = elu(mean_h att1 @ h1) ----
                with tc.tile_pool(name="pso", bufs=1, space="PSUM") as psop:
                    po = [
                        psop.tile([64, 384], F32, tag="po0", name="po0"),
                        psop.tile([64, 384], F32, tag="po1", name="po1"),
                    ]
                    for h in range(H):
                        for q in range(4):
                            lhs = at1T[q][:, :].rearrange(
                                "p (b c r) -> p b c r", b=NBLK, c=4
                            )[:, :, :, 12 + h : 13 + h]
                            for half in range(2):
                                nc.tensor.matmul(
                                    po[half][:, :], lhs,
                                    h1f[q][
                                        :, D * h + 384 * half : D * h + 384 * (half + 1)
                                    ],
                                    start=(h == 0 and q == 0),
                                    stop=(h == H - 1 and q == 3),
                                )
                    op = work.tile([64, D], F32, tag="op")
                    for half in range(2):
                        nc.vector.tensor_scalar(
                            out=op[:, 384 * half : 384 * (half + 1)],
                            in0=po[half][:, :],
                            scalar1=1.0 / H, scalar2=None, op0=MULT,
                        )
                omin = work.tile([64, D], F32, tag="omin")
                nc.vector.tensor_scalar(
                    out=omin[:, :], in0=op[:, :], scalar1=0.0, scalar2=None,
                    op0=mybir.AluOpType.min,
                )
                oexp = work.tile([64, D], F32, tag="oexp")
                nc.scalar.activation(oexp[:, :], omin[:, :], AF.Exp)
                omax = work.tile([64, D], F32, tag="omax")
                nc.vector.tensor_scalar(
                    out=omax[:, :], in0=op[:, :], scalar1=0.0, scalar2=None,
                    op0=mybir.AluOpType.max,
                )
                ofin = work.tile([64, D], F32, tag="ofin")
                nc.vector.scalar_tensor_tensor(
                    out=ofin[:, :], in0=oexp[:, :], scalar=-1.0, in1=omax[:, :],
                    op0=ADD, op1=ADD,
                )
                nc.scalar.dma_start(out=out_d[:, :], in_=ofin[:, :])

    nc.compile()
    return nc


def _fold_weights(We, W, a, F_):
    We = We.astype(np.float64)
    W = W.astype(np.float64)
    a = a.astype(np.float64)
    a1, a2, a3 = a[:, :F_], a[:, F_ : 2 * F_], a[:, 2 * F_ :]
    v = np.einsum("hei,hif,hf->he", We, W, a3)
    usrc = np.einsum("hif,hf->hi", W, a1)
    udst = np.einsum("hif,hf->hi", W, a2)
    return v, usrc, udst


def _to_ktile(mat):
    """[768, C] -> [128, KT*C] with the KT k-tiles side by side."""
    k, c = mat.shape
    assert k == D
    return np.ascontiguousarray(
        mat.reshape(KT, 128, c).transpose(1, 0, 2).reshape(128, KT * c)
    )


def kernel(**inputs):
    global _COMPILED
    x = np.asarray(inputs["x"], dtype=np.float32)
    adj = np.asarray(inputs["adj"])
    e = np.asarray(inputs["e"], dtype=np.float32)
    W0 = np.asarray(inputs["W0"], dtype=np.float32)
    a0 = np.asarray(inputs["a0"], dtype=np.float32)
    W1 = np.asarray(inputs["W1"], dtype=np.float32)
    a1_ = np.asarray(inputs["a1"], dtype=np.float32)
    We0 = np.asarray(inputs["We0"], dtype=np.float32)
    We1 = np.asarray(inputs["We1"], dtype=np.float32)

    v0, _, _ = _fold_weights(We0, W0, a0, F0)
    v1, u1src, u1dst = _fold_weights(We1, W1, a1_, D)
    V = np.concatenate([v0, v1], axis=0).T.astype(np.float32)        # [768, 24]
    U1 = np.concatenate([u1src, u1dst], axis=0).T.astype(np.float32)  # [768, 24]

    h0h = np.einsum("ni,hif->hnf", x.astype(np.float64), W0.astype(np.float64))
    s_src0 = np.einsum("hnf,hf->hn", h0h, a0[:, :F0].astype(np.float64))
    s_dst0 = np.einsum("hnf,hf->hn", h0h, a0[:, F0 : 2 * F0].astype(np.float64))
    maskadd = (adj.astype(np.float32) - 1.0) * 9e15                   # 0 or -9e15

    xT_bf = _to_ktile(np.ascontiguousarray(x.T)).astype(BF16)
    w0r_bf = _to_ktile(W0.transpose(1, 0, 2).reshape(D, H * F0)).astype(BF16)
    W1r = W1.transpose(1, 0, 2).reshape(D, H * D)
    Vp = np.zeros((D, 32), np.float32)
    Vp[:, :24] = V
    v_bf = _to_ktile(Vp).astype(BF16)
    u1_bf = _to_ktile(U1).astype(BF16)
    ident = np.eye(128, dtype=np.float32).astype(BF16)
    eT = np.ascontiguousarray(e.transpose(0, 2, 1)).astype(BF16)      # [N, D, N]

    in_maps = []
    for c in range(NCORES):
        ha0 = np.zeros((NBLK, 128, N), dtype=np.float32)
        mk1 = np.zeros((NBLK, 128, N), dtype=np.float32)
        for b in range(NBLK):
            for cc in range(4):
                i = NPC * c + 4 * b + cc
                ha0[b, 32 * cc : 32 * cc + 12, :] = (
                    s_dst0 + s_src0[:, i : i + 1] + maskadd[i : i + 1, :]
                )
                mk1[b, 32 * cc + 12 : 32 * cc + 24, :] = maskadd[i : i + 1, :]
        w1s_bf = _to_ktile(
            np.ascontiguousarray(W1r[:, W1COLS * c : W1COLS * (c + 1)])
        ).astype(BF16)
        in_maps.append(
            {
                "eT": eT[NPC * c : NPC * (c + 1)],
                "xT": xT_bf,
                "w0r": w0r_bf,
                "w1s": w1s_bf,
                "vw": v_bf,
                "u1": u1_bf,
                "ha0": ha0.astype(BF16),
                "mk1": mk1.astype(BF16),
                "ident": ident,
            }
        )

    if _COMPILED is None:
        _COMPILED = _build_nc()
    nc = _COMPILED

    res = run_bass_kernel_spmd(nc, in_maps, list(range(NCORES)))
    global _LAST_RESULTS
    _LAST_RESULTS = res.results
    out = np.concatenate([res.results[c]["out"] for c in range(NCORES)], axis=0)
    return out.astype(np.float32)


if __name__ == "__main__":
    import reference

    inputs = {k: np.asarray(v) for k, v in reference.setup_inputs().items()}
    got = kernel(**inputs)
    print("output shape:", got.shape, got.dtype)

